# revision 34
# baseline (speedup 1.0000x reference)
"""Trainium2 Bass kernel: grouped-experts SwiGLU MLP with mid-RMSNorm.

Expert-parallel across 8 NeuronCores: core e computes expert e's token
block (tokens are pre-sorted by expert).  Host gathers each expert's
rows into a zero-padded [C, D] buffer, ships transposed activations and
weights, and scatters the per-core outputs back to flat token order.

Per-core math (all fp16 in / fp32 accumulate):
    h1 = x @ w1^T ; h3 = x @ w3^T          # [C, F]
    h  = silu(h1) * h3
    h  = h * rsqrt(mean(h^2) + eps)        # RMSNorm (scale folded to out)
    out = (h * mid_w) @ w2^T               # mid_w folded into w2 on host

DMA notes: per-queue bandwidth is a fraction of the ~358 GB/s HBM
aggregate, so the streams are spread over the three DMA-capable queues
(sync/gpsimd/scalar) with the first f-block's weights issued in
consumption order, greedily balanced.  All host-side tensors are
pre-arranged so every DMA slab is >=2KB-contiguous per partition.
"""

import sys

sys.path.insert(0, "/opt/trn_rl_repo")

import numpy as np
from contextlib import ExitStack

import os

import concourse.bass as bass
import concourse.tile as tile
from concourse import bacc, mybir
from concourse.masks import make_identity

P = 128
T = 4096
D = 2048
F = 1024
E = 8
NB = 512  # matmul moving-dim block (one PSUM bank of fp32)
EPS = 1e-6
F32 = mybir.dt.float32
F16 = mybir.dt.float16
ACTF = mybir.ActivationFunctionType

_PROGRAM_CACHE: dict[int, object] = {}
LAST_RESULTS = None  # test harness reads per-core outputs from here


def _run(nc, in_maps):
    """Execute the compiled program on the 8 axon-tunneled cores.

    If KERNEL_NTFF_DIR is set, wrap the execute in the axon NTFF profile
    hook so device profiles land there (test harness use only).
    """
    from concourse import bass2jax

    ntff_dir = os.environ.get("KERNEL_NTFF_DIR")
    if ntff_dir:
        if "/root/.axon_site" not in sys.path:
            sys.path.insert(0, "/root/.axon_site")
        from trn_agent_boot.trn_boot import _ntff_profile_via_ctypes

        hook = _ntff_profile_via_ctypes("/opt/axon/libaxon_pjrt.so")
        ids = [
            int(x) for x in os.environ.get("KERNEL_NTFF_CORES", "0").split(",")
        ]
        if hook is not None:
            with hook(ntff_dir, ids):
                return bass2jax.run_bass_via_pjrt(nc, in_maps, n_cores=len(in_maps))
    return bass2jax.run_bass_via_pjrt(nc, in_maps, n_cores=len(in_maps))


def _tile_stats_and_transpose(nc, qpool, ps_t, h_tiles, ht_tiles, ssq_all, ident, t):
    KF = len(ht_tiles)
    hsq = qpool.tile([P, h_tiles[t].shape[1]], F32, tag="hsq", name=f"hsq{t}")
    nc.scalar.activation(
        hsq[:], h_tiles[t][:], ACTF.Square, accum_out=ssq_all[:, t : t + 1]
    )
    for fc in range(KF):
        pst = ps_t.tile([P, P], F16, tag="tp", name=f"pst{t}_{fc}")
        nc.tensor.transpose(pst[:], h_tiles[t][:, fc * P : (fc + 1) * P], ident[:])
        nc.vector.tensor_copy(ht_tiles[fc][:, t * P : (t + 1) * P], pst[:])


def _build_program(C: int):
    """Build + compile the single-core SPMD program for C padded rows."""
    NT = C // P  # token tiles per core
    KD = D // P  # 16 contraction chunks for mm1
    KF = F // P  # 8 contraction chunks for mm2
    FB = F // NB  # 2 f-blocks
    DB = D // NB  # 4 d-blocks

    nc = bacc.Bacc(
        "TRN2",
        target_bir_lowering=False,
        debug=False,
        enable_asserts=False,
        num_devices=E,
    )
    xT_d = nc.dram_tensor("xT", [P, NT, KD, P], F16, kind="ExternalInput").ap()
    w1_d = nc.dram_tensor("w1t", [P, FB, KD, NB], F16, kind="ExternalInput").ap()
    w3_d = nc.dram_tensor("w3t", [P, FB, KD, NB], F16, kind="ExternalInput").ap()
    w2_d = nc.dram_tensor("w2t", [P, DB, KF, NB], F16, kind="ExternalInput").ap()
    out_d = nc.dram_tensor("out", [C, D], F16, kind="ExternalOutput").ap()

    with tile.TileContext(nc) as tc, ExitStack() as ctx:
        singles = ctx.enter_context(tc.tile_pool(name="singles", bufs=1))
        xpool = ctx.enter_context(tc.tile_pool(name="x", bufs=1))
        w1pool = ctx.enter_context(tc.tile_pool(name="w1", bufs=2))
        w3pool = ctx.enter_context(tc.tile_pool(name="w3", bufs=2))
        w2pool = ctx.enter_context(tc.tile_pool(name="w2", bufs=4))
        hpool = ctx.enter_context(tc.tile_pool(name="h", bufs=1))
        htpool = ctx.enter_context(tc.tile_pool(name="ht", bufs=1))
        spool = ctx.enter_context(tc.tile_pool(name="scr", bufs=2))
        qpool = ctx.enter_context(tc.tile_pool(name="sq", bufs=1))
        opool = ctx.enter_context(tc.tile_pool(name="o", bufs=8))
        stat = ctx.enter_context(tc.tile_pool(name="stat", bufs=1))
        ps_h = ctx.enter_context(tc.tile_pool(name="psh", bufs=2, space="PSUM"))
        ps_t = ctx.enter_context(tc.tile_pool(name="pst", bufs=2, space="PSUM"))
        ps_o = ctx.enter_context(tc.tile_pool(name="pso", bufs=2, space="PSUM"))

        ident = singles.tile([P, P], F16)
        make_identity(nc, ident[:])
        eps_t = singles.tile([P, 1], F32, name="epsT")
        nc.gpsimd.memset(eps_t[:], EPS)

        xt = xpool.tile([P, NT, KD, P], F16)

        # ---- prologue DMA schedule: fb0 weights + x tiles in consumption
        # order, greedily balanced over the three DMA-capable queues
        # (~0.1 TB/s each).  The scalar queue is capped so its phase-A
        # sigmoids are not pushed past the PSUM-release deadline.
        queues = [nc.sync, nc.gpsimd, nc.scalar]
        qload = [0, 0, 0]
        SCALAR_CAP = 1_700_000
        # measured relative queue rates: sync (HWDGE) is ~2-3x faster than
        # the software-DGE gpsimd queue and the scalar queue
        RATE = [1.0, 0.45, 0.35]

        def issue(dst, src, nbytes, qi=None):
            if qi is None:
                elig = [0, 1] + ([2] if qload[2] < SCALAR_CAP else [])
                qi = min(elig, key=lambda i: (qload[i] + nbytes) / RATE[i])
            queues[qi].dma_start(dst, src)
            qload[qi] += nbytes

        w_tiles = {}
        for fb in range(FB):
            w_tiles[fb] = (
                w1pool.tile([P, KD, NB], F16, tag="w1", name=f"w1h{fb}"),
                w3pool.tile([P, KD, NB], F16, tag="w3", name=f"w3h{fb}"),
            )
        w2_tiles = [
            w2pool.tile([P, KF, NB], F16, tag="w2", name=f"w2b{db}")
            for db in range(DB)
        ]

        wunit2 = 2 * NB * P * 2
        wunit4 = 4 * NB * P * 2
        xunit = (KD // 2) * P * P * 2
        w1h0, w3h0 = w_tiles[0]
        # x tile 0 first on scalar, small leading chunk (gates the very
        # first matmuls), then the rest.
        issue(xt[:, 0, 0:4, :], xT_d[:, 0, 0:4, :], xunit // 2, qi=2)
        issue(xt[:, 0, 4:KD, :], xT_d[:, 0, 4:KD, :], 3 * xunit // 2, qi=2)
        # fb0 weights in consumption order: small k0/k1 units for an early
        # start, then 512KB units (fewer descriptors), greedily balanced.
        for j in range(2):
            ks = bass.ds(j * 2, 2)
            issue(w1h0[:, ks, :], w1_d[:, 0, ks, :], wunit2)
            issue(w3h0[:, ks, :], w3_d[:, 0, ks, :], wunit2)
        for j in range(1, KD // 4):
            ks = bass.ds(j * 4, 4)
            issue(w1h0[:, ks, :], w1_d[:, 0, ks, :], wunit4)
            issue(w3h0[:, ks, :], w3_d[:, 0, ks, :], wunit4)
        # remaining x tiles (whole-tile units)
        for t in range(1, NT):
            issue(xt[:, t], xT_d[:, t], 2 * xunit)
        # fb1 weights ride sync/gpsimd right behind the prologue (the
        # scalar queue must be free for phase-A sigmoids by then).
        for fb in range(1, FB):
            w1h, w3h = w_tiles[fb]
            for j in range(4):
                ks = bass.ds(j * 4, 4)
                issue(w1h[:, ks, :], w1_d[:, fb, ks, :], wunit4, qi=0)
                issue(w3h[:, ks, :], w3_d[:, fb, ks, :], wunit4, qi=1)
        # all w2 up front on sync/gpsimd too — they must not queue behind
        # phase C's output DMAs, and both queues are free by mid-phase-A.
        for db in range(DB):
            w2b = w2_tiles[db]
            issue(w2b[:, 0 : KF // 2, :], w2_d[:, db, 0 : KF // 2, :], wunit4, qi=0)
            issue(w2b[:, KF // 2 :, :], w2_d[:, db, KF // 2 :, :], wunit4, qi=1)

        h_tiles = [hpool.tile([P, F], F16, tag=f"h{t}", name=f"h{t}") for t in range(NT)]
        ht_tiles = [
            htpool.tile([P, C], F16, tag=f"ht{fc}", name=f"ht{fc}")
            for fc in range(KF)
        ]
        ssq_all = stat.tile([P, NT], F32, name="ssq_all")
        std_all = stat.tile([P, NT], F32, name="std_all")
        rstd_all = stat.tile([P, NT], F32, name="rstd_all")

        # ================= phase A: h1/h3 matmuls + swiglu =================
        # fb1 processes the LAST tile first so its phase-B work (fused one
        # iteration behind) lands early; the dangling tile is NT-2, which
        # phase C visits last.
        fb1_order = [NT - 1] + list(range(NT - 1)) if NT > 1 else [0]
        phc_order = fb1_order

        def _tile_done(t):
            # phase B (stats + transpose) then per-tile rstd, so phase C's
            # epilogue never waits on other tiles' statistics.
            _tile_stats_and_transpose(
                nc, qpool, ps_t, h_tiles, ht_tiles, ssq_all, ident, t
            )
            nc.scalar.activation(
                std_all[:, t : t + 1],
                ssq_all[:, t : t + 1],
                ACTF.Sqrt,
                bias=eps_t[:],
                scale=1.0 / F,
            )
            nc.vector.reciprocal(rstd_all[:, t : t + 1], std_all[:, t : t + 1])

        def _swiglu_epilogue(fb, t, ps1, ps3):
            s = spool.tile([P, NB], F32, tag="silu")
            nc.scalar.activation(s[:], ps1[:], ACTF.Sigmoid)
            hs = h_tiles[t][:, fb * NB : (fb + 1) * NB]
            nc.vector.tensor_mul(hs, s[:], ps1[:])
            nc.vector.tensor_mul(hs, hs, ps3[:])

        for fb in range(FB):
            w1h, w3h = w_tiles[fb]
            order = range(NT) if fb < FB - 1 else fb1_order
            for ti, t in enumerate(order):
                ps1 = ps_h.tile([P, NB], F32, tag="ps1")
                ps3 = ps_h.tile([P, NB], F32, tag="ps3")
                for k in range(KD):
                    xs = xt[:, t, k, :]
                    nc.tensor.matmul(
                        ps1[:], xs, w1h[:, k, :], start=(k == 0), stop=(k == KD - 1)
                    )
                    nc.tensor.matmul(
                        ps3[:], xs, w3h[:, k, :], start=(k == 0), stop=(k == KD - 1)
                    )
                _swiglu_epilogue(fb, t, ps1, ps3)
                if fb == FB - 1 and ti >= 1:
                    _tile_done(order[ti - 1])
        _tile_done(fb1_order[-1])

        # ================= phase C: out = hT.T @ w2T, scaled by rstd =======
        for db in range(DB):
            w2b = w2_tiles[db]
            for ti, t in enumerate(phc_order):
                pso = ps_o.tile([P, NB], F32, tag="po")
                for fc in range(KF):
                    nc.tensor.matmul(
                        pso[:],
                        ht_tiles[fc][:, t * P : (t + 1) * P],
                        w2b[:, fc, :],
                        start=(fc == 0),
                        stop=(fc == KF - 1),
                    )
                ob = opool.tile([P, NB], F16, tag="ob")
                nc.vector.tensor_scalar_mul(ob[:], pso[:], rstd_all[:, t : t + 1])
                oq = nc.sync if ti % 2 == 0 else nc.gpsimd
                oq.dma_start(
                    out_d[t * P : (t + 1) * P, db * NB : (db + 1) * NB], ob[:]
                )

    nc.compile()
    return nc


def _get_program(C: int):
    if C not in _PROGRAM_CACHE:
        _PROGRAM_CACHE[C] = _build_program(C)
    return _PROGRAM_CACHE[C]


def kernel(x, w1, w2, w3, mid_w, num_tokens_per_expert):
    global LAST_RESULTS
    x = np.ascontiguousarray(np.asarray(x, dtype=np.float32))
    w1 = np.asarray(w1, dtype=np.float32)
    w2 = np.asarray(w2, dtype=np.float32)
    w3 = np.asarray(w3, dtype=np.float32)
    mid_w = np.asarray(mid_w, dtype=np.float32)
    counts = np.asarray(num_tokens_per_expert).astype(np.int64)

    T_, D_ = x.shape
    E_, F_, _ = w1.shape
    Ccap = (T_ // E_) * 3 // 2  # reference static capacity (768)
    ends = np.cumsum(counts)
    starts = ends - counts
    eff = np.minimum(np.maximum(counts, 0), Ccap)  # rows actually computed

    C = int(max(P, -(-int(eff.max()) // P) * P))  # pad to token-tile multiple
    nc = _get_program(C)

    KD = D_ // P
    KF = F_ // P
    FB = F_ // NB
    DB = D_ // NB

    in_maps = []
    for e in range(E_):
        cnt = int(eff[e])
        s = int(starts[e])
        xg = np.zeros((C, D_), np.float32)
        if cnt > 0:
            rows = np.clip(s + np.arange(cnt), 0, T_ - 1)
            xg[:cnt] = x[rows]
        # [P, NT, KD, P] token-tile-major; every DMA slab contiguous.
        xg4 = xg.astype(np.float16).reshape(C // P, P, KD, P)
        w1p = w1[e].T.astype(np.float16).reshape(KD, P, FB, NB)
        w3p = w3[e].T.astype(np.float16).reshape(KD, P, FB, NB)
        w2p = (w2[e] * mid_w[None, :]).T.astype(np.float16).reshape(KF, P, DB, NB)
        in_maps.append(
            {
                "xT": np.ascontiguousarray(xg4.transpose(3, 0, 2, 1)),
                "w1t": np.ascontiguousarray(w1p.transpose(1, 2, 0, 3)),
                "w3t": np.ascontiguousarray(w3p.transpose(1, 2, 0, 3)),
                "w2t": np.ascontiguousarray(w2p.transpose(1, 2, 0, 3)),
            }
        )

    LAST_RESULTS = _run(nc, in_maps)
    outs = [LAST_RESULTS[e]["out"] for e in range(E_)]

    # scatter back to flat token order, mirroring the reference's clamping
    tok = np.arange(T_)
    eid = np.clip(np.searchsorted(ends, tok, side="right"), 0, E_ - 1)
    pos = tok - starts[eid]
    idx = np.minimum(pos, Ccap - 1)
    valid = (idx >= 0) & (idx < eff[eid])
    idx_safe = np.clip(idx, 0, C - 1)
    stacked = np.stack(outs, axis=0)  # [E, C, D]
    result = stacked[eid, idx_safe].astype(np.float32)
    result[~valid] = 0.0
    return result


# revision 35
# speedup vs baseline: 1.1863x; 1.1863x over previous
"""Trainium2 Bass kernel: grouped-experts SwiGLU MLP with mid-RMSNorm.

Expert-parallel across 8 NeuronCores: core e computes expert e's token
block (tokens are pre-sorted by expert).  Host gathers each expert's
rows into a zero-padded [C, D] buffer, ships transposed activations and
weights, and scatters the per-core outputs back to flat token order.

Per-core math (all fp16 in / fp32 accumulate):
    h1 = x @ w1^T ; h3 = x @ w3^T          # [C, F]
    h  = silu(h1) * h3
    h  = h * rsqrt(mean(h^2) + eps)        # RMSNorm (scale folded to out)
    out = (h * mid_w) @ w2^T               # mid_w folded into w2 on host

DMA notes: per-queue bandwidth is a fraction of the ~358 GB/s HBM
aggregate, so the streams are spread over the three DMA-capable queues
(sync/gpsimd/scalar) with the first f-block's weights issued in
consumption order, greedily balanced.  All host-side tensors are
pre-arranged so every DMA slab is >=2KB-contiguous per partition.
"""

import sys

sys.path.insert(0, "/opt/trn_rl_repo")

import numpy as np
from contextlib import ExitStack

import os

import concourse.bass as bass
import concourse.tile as tile
from concourse import bacc, mybir
from concourse.masks import make_identity

P = 128
T = 4096
D = 2048
F = 1024
E = 8
NB = 512  # matmul moving-dim block (one PSUM bank of fp32)
EPS = 1e-6
F32 = mybir.dt.float32
F16 = mybir.dt.float16
ACTF = mybir.ActivationFunctionType

_PROGRAM_CACHE: dict[int, object] = {}
LAST_RESULTS = None  # test harness reads per-core outputs from here


def _run(nc, in_maps):
    """Execute the compiled program on the 8 axon-tunneled cores.

    If KERNEL_NTFF_DIR is set, wrap the execute in the axon NTFF profile
    hook so device profiles land there (test harness use only).
    """
    from concourse import bass2jax

    ntff_dir = os.environ.get("KERNEL_NTFF_DIR")
    if ntff_dir:
        if "/root/.axon_site" not in sys.path:
            sys.path.insert(0, "/root/.axon_site")
        from trn_agent_boot.trn_boot import _ntff_profile_via_ctypes

        hook = _ntff_profile_via_ctypes("/opt/axon/libaxon_pjrt.so")
        ids = [
            int(x) for x in os.environ.get("KERNEL_NTFF_CORES", "0").split(",")
        ]
        if hook is not None:
            with hook(ntff_dir, ids):
                return bass2jax.run_bass_via_pjrt(nc, in_maps, n_cores=len(in_maps))
    return bass2jax.run_bass_via_pjrt(nc, in_maps, n_cores=len(in_maps))


def _tile_stats_and_transpose(nc, qpool, ps_t, h_tiles, ht_tiles, ssq_all, ident, t):
    KF = len(ht_tiles)
    hsq = qpool.tile([P, h_tiles[t].shape[1]], F32, tag="hsq", name=f"hsq{t}")
    nc.scalar.activation(
        hsq[:], h_tiles[t][:], ACTF.Square, accum_out=ssq_all[:, t : t + 1]
    )
    for fc in range(KF):
        pst = ps_t.tile([P, P], F16, tag="tp", name=f"pst{t}_{fc}")
        nc.tensor.transpose(pst[:], h_tiles[t][:, fc * P : (fc + 1) * P], ident[:])
        nc.vector.tensor_copy(ht_tiles[fc][:, t * P : (t + 1) * P], pst[:])


def _build_program(C: int):
    """Build + compile the single-core SPMD program for C padded rows."""
    NT = C // P  # token tiles per core
    KD = D // P  # 16 contraction chunks for mm1
    KF = F // P  # 8 contraction chunks for mm2
    FB = F // NB  # 2 f-blocks
    DB = D // NB  # 4 d-blocks

    nc = bacc.Bacc(
        "TRN2",
        target_bir_lowering=False,
        debug=False,
        enable_asserts=False,
        num_devices=E,
    )
    xT_d = nc.dram_tensor("xT", [P, NT, KD, P], F16, kind="ExternalInput").ap()
    w1_d = nc.dram_tensor("w1t", [P, FB, KD, NB], F16, kind="ExternalInput").ap()
    w3_d = nc.dram_tensor("w3t", [P, FB, KD, NB], F16, kind="ExternalInput").ap()
    w2_d = nc.dram_tensor("w2t", [P, DB, KF, NB], F16, kind="ExternalInput").ap()
    out_d = nc.dram_tensor("out", [C, D], F16, kind="ExternalOutput").ap()

    with tile.TileContext(nc) as tc, ExitStack() as ctx:
        singles = ctx.enter_context(tc.tile_pool(name="singles", bufs=1))
        xpool = ctx.enter_context(tc.tile_pool(name="x", bufs=1))
        w1pool = ctx.enter_context(tc.tile_pool(name="w1", bufs=2))
        w3pool = ctx.enter_context(tc.tile_pool(name="w3", bufs=2))
        w2pool = ctx.enter_context(tc.tile_pool(name="w2", bufs=4))
        hpool = ctx.enter_context(tc.tile_pool(name="h", bufs=1))
        htpool = ctx.enter_context(tc.tile_pool(name="ht", bufs=1))
        spool = ctx.enter_context(tc.tile_pool(name="scr", bufs=2))
        qpool = ctx.enter_context(tc.tile_pool(name="sq", bufs=1))
        opool = ctx.enter_context(tc.tile_pool(name="o", bufs=8))
        stat = ctx.enter_context(tc.tile_pool(name="stat", bufs=1))
        ps_h = ctx.enter_context(tc.tile_pool(name="psh", bufs=2, space="PSUM"))
        ps_t = ctx.enter_context(tc.tile_pool(name="pst", bufs=2, space="PSUM"))
        ps_o = ctx.enter_context(tc.tile_pool(name="pso", bufs=2, space="PSUM"))

        ident = singles.tile([P, P], F16)
        make_identity(nc, ident[:])
        eps_t = singles.tile([P, 1], F32, name="epsT")
        nc.gpsimd.memset(eps_t[:], EPS)

        xt = xpool.tile([P, NT, KD, P], F16)

        # ---- prologue DMA schedule: fb0 weights + x tiles in consumption
        # order, greedily balanced over the three DMA-capable queues
        # (~0.1 TB/s each).  The scalar queue is capped so its phase-A
        # sigmoids are not pushed past the PSUM-release deadline.
        queues = [nc.sync, nc.gpsimd, nc.scalar]
        qload = [0, 0, 0]
        SCALAR_CAP = 1_700_000

        def issue(dst, src, nbytes, qi=None):
            if qi is None:
                elig = [0, 1] + ([2] if qload[2] < SCALAR_CAP else [])
                qi = min(elig, key=lambda i: qload[i])
            queues[qi].dma_start(dst, src)
            qload[qi] += nbytes

        w_tiles = {}
        for fb in range(FB):
            w_tiles[fb] = (
                w1pool.tile([P, KD, NB], F16, tag="w1", name=f"w1h{fb}"),
                w3pool.tile([P, KD, NB], F16, tag="w3", name=f"w3h{fb}"),
            )
        w2_tiles = [
            w2pool.tile([P, KF, NB], F16, tag="w2", name=f"w2b{db}")
            for db in range(DB)
        ]

        wunit2 = 2 * NB * P * 2
        wunit4 = 4 * NB * P * 2
        xunit = (KD // 2) * P * P * 2
        w1h0, w3h0 = w_tiles[0]
        # x tile 0 first on scalar, small leading chunk (gates the very
        # first matmuls), then the rest.
        issue(xt[:, 0, 0:4, :], xT_d[:, 0, 0:4, :], xunit // 2, qi=2)
        issue(xt[:, 0, 4:KD, :], xT_d[:, 0, 4:KD, :], 3 * xunit // 2, qi=2)
        # fb0 weights in consumption order: small k0/k1 units for an early
        # start, then 512KB units (fewer descriptors), greedily balanced.
        for j in range(2):
            ks = bass.ds(j * 2, 2)
            issue(w1h0[:, ks, :], w1_d[:, 0, ks, :], wunit2)
            issue(w3h0[:, ks, :], w3_d[:, 0, ks, :], wunit2)
        for j in range(1, KD // 4):
            ks = bass.ds(j * 4, 4)
            issue(w1h0[:, ks, :], w1_d[:, 0, ks, :], wunit4)
            issue(w3h0[:, ks, :], w3_d[:, 0, ks, :], wunit4)
        # remaining x tiles (whole-tile units)
        for t in range(1, NT):
            issue(xt[:, t], xT_d[:, t], 2 * xunit)
        # fb1 weights ride sync/gpsimd right behind the prologue (the
        # scalar queue must be free for phase-A sigmoids by then).
        for fb in range(1, FB):
            w1h, w3h = w_tiles[fb]
            for j in range(4):
                ks = bass.ds(j * 4, 4)
                issue(w1h[:, ks, :], w1_d[:, fb, ks, :], wunit4, qi=0)
                issue(w3h[:, ks, :], w3_d[:, fb, ks, :], wunit4, qi=1)
        # all w2 up front on sync/gpsimd too — they must not queue behind
        # phase C's output DMAs, and both queues are free by mid-phase-A.
        for db in range(DB):
            w2b = w2_tiles[db]
            issue(w2b[:, 0 : KF // 2, :], w2_d[:, db, 0 : KF // 2, :], wunit4, qi=0)
            issue(w2b[:, KF // 2 :, :], w2_d[:, db, KF // 2 :, :], wunit4, qi=1)

        h_tiles = [hpool.tile([P, F], F16, tag=f"h{t}", name=f"h{t}") for t in range(NT)]
        ht_tiles = [
            htpool.tile([P, C], F16, tag=f"ht{fc}", name=f"ht{fc}")
            for fc in range(KF)
        ]
        ssq_all = stat.tile([P, NT], F32, name="ssq_all")
        std_all = stat.tile([P, NT], F32, name="std_all")
        rstd_all = stat.tile([P, NT], F32, name="rstd_all")

        # ================= phase A: h1/h3 matmuls + swiglu =================
        # fb1 processes the LAST tile first so its phase-B work (fused one
        # iteration behind) lands early; the dangling tile is NT-2, which
        # phase C visits last.
        fb1_order = [NT - 1] + list(range(NT - 1)) if NT > 1 else [0]
        phc_order = fb1_order

        def _tile_done(t):
            # phase B (stats + transpose) then per-tile rstd, so phase C's
            # epilogue never waits on other tiles' statistics.
            _tile_stats_and_transpose(
                nc, qpool, ps_t, h_tiles, ht_tiles, ssq_all, ident, t
            )
            nc.scalar.activation(
                std_all[:, t : t + 1],
                ssq_all[:, t : t + 1],
                ACTF.Sqrt,
                bias=eps_t[:],
                scale=1.0 / F,
            )
            nc.vector.reciprocal(rstd_all[:, t : t + 1], std_all[:, t : t + 1])

        def _swiglu_epilogue(fb, t, ps1, ps3):
            s = spool.tile([P, NB], F32, tag="silu")
            nc.scalar.activation(s[:], ps1[:], ACTF.Sigmoid)
            hs = h_tiles[t][:, fb * NB : (fb + 1) * NB]
            nc.vector.tensor_mul(hs, s[:], ps1[:])
            nc.vector.tensor_mul(hs, hs, ps3[:])

        for fb in range(FB):
            w1h, w3h = w_tiles[fb]
            order = range(NT) if fb < FB - 1 else fb1_order
            for ti, t in enumerate(order):
                ps1 = ps_h.tile([P, NB], F32, tag="ps1")
                ps3 = ps_h.tile([P, NB], F32, tag="ps3")
                for k in range(KD):
                    xs = xt[:, t, k, :]
                    nc.tensor.matmul(
                        ps1[:], xs, w1h[:, k, :], start=(k == 0), stop=(k == KD - 1)
                    )
                    nc.tensor.matmul(
                        ps3[:], xs, w3h[:, k, :], start=(k == 0), stop=(k == KD - 1)
                    )
                _swiglu_epilogue(fb, t, ps1, ps3)
                if fb == FB - 1 and ti >= 1:
                    _tile_done(order[ti - 1])
        _tile_done(fb1_order[-1])

        # ================= phase C: out = hT.T @ w2T, scaled by rstd =======
        for db in range(DB):
            w2b = w2_tiles[db]
            for ti, t in enumerate(phc_order):
                pso = ps_o.tile([P, NB], F32, tag="po")
                for fc in range(KF):
                    nc.tensor.matmul(
                        pso[:],
                        ht_tiles[fc][:, t * P : (t + 1) * P],
                        w2b[:, fc, :],
                        start=(fc == 0),
                        stop=(fc == KF - 1),
                    )
                ob = opool.tile([P, NB], F16, tag="ob")
                nc.vector.tensor_scalar_mul(ob[:], pso[:], rstd_all[:, t : t + 1])
                oq = nc.sync if ti % 2 == 0 else nc.gpsimd
                oq.dma_start(
                    out_d[t * P : (t + 1) * P, db * NB : (db + 1) * NB], ob[:]
                )

    nc.compile()
    return nc


def _get_program(C: int):
    if C not in _PROGRAM_CACHE:
        _PROGRAM_CACHE[C] = _build_program(C)
    return _PROGRAM_CACHE[C]


def kernel(x, w1, w2, w3, mid_w, num_tokens_per_expert):
    global LAST_RESULTS
    x = np.ascontiguousarray(np.asarray(x, dtype=np.float32))
    w1 = np.asarray(w1, dtype=np.float32)
    w2 = np.asarray(w2, dtype=np.float32)
    w3 = np.asarray(w3, dtype=np.float32)
    mid_w = np.asarray(mid_w, dtype=np.float32)
    counts = np.asarray(num_tokens_per_expert).astype(np.int64)

    T_, D_ = x.shape
    E_, F_, _ = w1.shape
    Ccap = (T_ // E_) * 3 // 2  # reference static capacity (768)
    ends = np.cumsum(counts)
    starts = ends - counts
    eff = np.minimum(np.maximum(counts, 0), Ccap)  # rows actually computed

    C = int(max(P, -(-int(eff.max()) // P) * P))  # pad to token-tile multiple
    nc = _get_program(C)

    KD = D_ // P
    KF = F_ // P
    FB = F_ // NB
    DB = D_ // NB

    in_maps = []
    for e in range(E_):
        cnt = int(eff[e])
        s = int(starts[e])
        xg = np.zeros((C, D_), np.float32)
        if cnt > 0:
            rows = np.clip(s + np.arange(cnt), 0, T_ - 1)
            xg[:cnt] = x[rows]
        # [P, NT, KD, P] token-tile-major; every DMA slab contiguous.
        xg4 = xg.astype(np.float16).reshape(C // P, P, KD, P)
        w1p = w1[e].T.astype(np.float16).reshape(KD, P, FB, NB)
        w3p = w3[e].T.astype(np.float16).reshape(KD, P, FB, NB)
        w2p = (w2[e] * mid_w[None, :]).T.astype(np.float16).reshape(KF, P, DB, NB)
        in_maps.append(
            {
                "xT": np.ascontiguousarray(xg4.transpose(3, 0, 2, 1)),
                "w1t": np.ascontiguousarray(w1p.transpose(1, 2, 0, 3)),
                "w3t": np.ascontiguousarray(w3p.transpose(1, 2, 0, 3)),
                "w2t": np.ascontiguousarray(w2p.transpose(1, 2, 0, 3)),
            }
        )

    LAST_RESULTS = _run(nc, in_maps)
    outs = [LAST_RESULTS[e]["out"] for e in range(E_)]

    # scatter back to flat token order, mirroring the reference's clamping
    tok = np.arange(T_)
    eid = np.clip(np.searchsorted(ends, tok, side="right"), 0, E_ - 1)
    pos = tok - starts[eid]
    idx = np.minimum(pos, Ccap - 1)
    valid = (idx >= 0) & (idx < eff[eid])
    idx_safe = np.clip(idx, 0, C - 1)
    stacked = np.stack(outs, axis=0)  # [E, C, D]
    result = stacked[eid, idx_safe].astype(np.float32)
    result[~valid] = 0.0
    return result
